# revision 11
# baseline (speedup 1.0000x reference)
"""Distributed multi-head attention for 8 TRN2 NeuronCores — v2.

Problem: x[2,2048,1024] -> QKV proj (w_qkv[3072,1024]) -> 16-head SDPA ->
out proj (w_proj[1024,1024] + b_proj) -> [2,2048,1024].

Sharding: 2 heads per core (head-parallel over all 8 cores; both batches on
every core). Per core:
  Phase A: qT/kT [128(=2 heads x 64d), 4096] and V-natural [4096, 128] from
           x @ w_qkv_shard.T; V via PE transpose of vT.
  Phase B: transposed-score attention per (batch, 512-query chunk) u:
           S^T[m,n] = kT.T @ qT (row-tiled K=64 matmul pairs),
           P = exp(S*scale), O^T_ext[65,n] = [V|1].T @ P^T accumulated over
           key tiles in PSUM (row 64 = softmax denominator). Normalized
           output for both heads lands in o_n2[128ch, 512tok].
  Fused out-proj: partial[512tok, 1024f] = o_n2.T @ wpT_shard (my 128
           channels only; K=128, no accumulation) -> fp16 -> rs_in[u].
  Per-unit ReduceScatter #u sums partials over cores and scatters rows:
           core c receives final out rows [u*512 + c*64 .. +64) (minus bias).
           The 8 RS ops overlap with subsequent attention compute.
  Post-RS: add bias, write fp32 out rows. Host reorders rows.
"""
import sys, os, types
import numpy as np

if "/opt/trn_rl_repo" not in sys.path and os.path.isdir("/opt/trn_rl_repo"):
    sys.path.append("/opt/trn_rl_repo")

import concourse.bass as bass
import concourse.mybir as mybir
import concourse.tile as tile
from concourse import bacc
from concourse.bass_utils import run_bass_kernel_spmd

F32 = mybir.dt.float32
F16 = mybir.dt.float16
BF16 = mybir.dt.bfloat16
EXP = mybir.ActivationFunctionType.Exp

NCORES = 8
B, N, C, H, D = 2, 2048, 1024, 16, 64
NT = B * N          # 4096 flat tokens
KT = C // 128       # 8 contraction tiles of 128
QC = 512            # query-chunk width (one PSUM bank)
NU = NT // QC       # 8 (batch, qchunk) units == RS count
NMT = N // 128      # 16 key tiles per batch
SCALE = 1.0 / 8.0   # 1/sqrt(D)
GRP = 2             # score banks per exp call
XCH = 1024          # x load chunk width

TRACE = False
LAST_EXEC_NS = None

_NC = None


def _install_ntff_hook():
    if "antenv.axon_hooks" in sys.modules:
        return
    try:
        import antenv
        from trn_agent_boot.trn_boot import _ntff_profile_via_ctypes
        mod = types.ModuleType("antenv.axon_hooks")
        _hook = [None]
        mod.set_axon_ntff_profile_hook = lambda h: _hook.__setitem__(0, h)
        mod.get_axon_ntff_profile_hook = lambda: _hook[0]
        sys.modules["antenv.axon_hooks"] = mod
        antenv.axon_hooks = mod
        mod.set_axon_ntff_profile_hook(
            _ntff_profile_via_ctypes("/opt/axon/libaxon_pjrt.so"))
    except Exception:
        pass


def _build():
    nc = bacc.Bacc("TRN2", target_bir_lowering=False, debug=False,
                   num_devices=NCORES)
    xT_ext = nc.dram_tensor("xT", [C, NT], BF16, kind="ExternalInput").ap()
    wT_ext = nc.dram_tensor("wT", [C, 384], BF16, kind="ExternalInput").ap()
    wpT_ext = nc.dram_tensor("wpT", [128, C], BF16, kind="ExternalInput").ap()
    bias_ext = nc.dram_tensor("bias", [1, C], F32, kind="ExternalInput").ap()
    idn_ext = nc.dram_tensor("idn", [128, 128], BF16, kind="ExternalInput").ap()
    out_ext = nc.dram_tensor("out", [NT // NCORES, C], F32,
                             kind="ExternalOutput").ap()
    rs_in = [nc.dram_tensor(f"rs_in{u}", [QC, C], F16).ap()
             for u in range(NU)]
    rs_out = [nc.dram_tensor(f"rs_out{u}", [64, C], F16).ap()
              for u in range(NU)]

    xT_v = xT_ext.rearrange("(kt p) n -> p kt n", p=128)
    wT_v = wT_ext.rearrange("(kt p) f -> p kt f", p=128)

    with tile.TileContext(nc) as tc:
        with (
            tc.tile_pool(name="const", bufs=1) as cpool,
            tc.tile_pool(name="resid", bufs=1) as rpool,
        ):
            wT_sb = cpool.tile([128, KT, 384], BF16)
            for kt in range(KT):
                nc.sync.dma_start(wT_sb[:, kt, :], wT_v[:, kt, :])
            idn = cpool.tile([128, 128], BF16)
            nc.sync.dma_start(idn[:], idn_ext[:])
            wp_sb = cpool.tile([128, C], BF16)
            nc.sync.dma_start(wp_sb[:], wpT_ext[:])
            bias_sb = cpool.tile([1, C], F32)
            nc.sync.dma_start(bias_sb[:], bias_ext[:])
            bias_bc = cpool.tile([64, C], F32)
            nc.gpsimd.partition_broadcast(bias_bc[:], bias_sb[:])

            qT_sb = rpool.tile([128, NT], BF16)
            kT_sb = rpool.tile([128, NT], BF16)
            v_sb = rpool.tile([128, NT // 128, 130], BF16)
            nc.gpsimd.memset(v_sb[:, :, 64], 1.0)
            nc.gpsimd.memset(v_sb[:, :, 129], 1.0)
            # unnormalized attention outputs + denominator row: block (u, h)
            # at [0:65, u*2+h, :]; row 64 is the softmax denominator
            stage = rpool.tile([65, 2 * NU, QC], F32)

            def qkv_groups(vpool, apsum, tpsum, x_tiles, bat, psum_tag):
                """Yield one closure per QKV matmul-group (8 accumulating
                matmuls + PSUM evacuation; the v-feature groups also emit
                the PE transposes building V-natural)."""
                for nch2 in range(N // XCH):
                    x_t = x_tiles[bat * (N // XCH) + nch2]
                    for hw in range(XCH // QC):
                        ncol = bat * N + nch2 * XCH + hw * QC
                        for ft in range(3):
                            def emit(ncol=ncol, hw=hw, ft=ft, x_t=x_t):
                                xs = x_t[:, :, hw * QC:(hw + 1) * QC]
                                ps = apsum.tile([128, QC], F32, tag=psum_tag,
                                                name=f"qkv_{ncol}_{ft}")
                                for kt in range(KT):
                                    nc.tensor.matmul(
                                        ps[:],
                                        wT_sb[:, kt, ft * 128:(ft + 1) * 128],
                                        xs[:, kt, :],
                                        start=(kt == 0), stop=(kt == KT - 1))
                                if ft == 0:
                                    nc.vector.tensor_copy(
                                        out=qT_sb[:, ncol:ncol + QC],
                                        in_=ps[:])
                                elif ft == 1:
                                    nc.vector.tensor_copy(
                                        out=kT_sb[:, ncol:ncol + QC],
                                        in_=ps[:])
                                else:
                                    vt = vpool.tile([128, QC], BF16, tag="vt",
                                                    name=f"vt_{ncol}")
                                    nc.vector.tensor_copy(out=vt[:],
                                                          in_=ps[:])
                                    mtg0 = ncol // 128
                                    trp = tpsum.tile(
                                        [128, 4, 128], BF16, tag="tr",
                                        name=f"tr_{mtg0}")
                                    for t in range(4):
                                        nc.tensor.transpose(
                                            trp[:, t, :],
                                            vt[:, t * 128:(t + 1) * 128],
                                            idn[:])
                                    nc.vector.tensor_copy(
                                        out=v_sb[:, mtg0:mtg0 + 4, 0:64],
                                        in_=trp[:, :, 0:64])
                                    nc.vector.tensor_copy(
                                        out=v_sb[:, mtg0:mtg0 + 4, 65:129],
                                        in_=trp[:, :, 64:128])
                            yield emit

            def proj_closure(u, o_n2, auxpsum, projpool):
                """Fused out-projection for unit u: partial[tok,f] from my
                128 channels; fp16 to rs_in[u]; then the ReduceScatter."""
                def emit():
                    for tt in range(4):
                        for half in range(2):
                            pp = auxpsum.tile([128, QC], F32, tag="pp",
                                              name=f"pp_{u}_{tt}_{half}")
                            nc.tensor.matmul(
                                pp[:],
                                o_n2[:, tt * 128:(tt + 1) * 128],
                                wp_sb[:, half * QC:(half + 1) * QC],
                                start=True, stop=True)
                            pt = projpool.tile([128, QC], F16, tag="pt",
                                               name=f"pt_{u}_{tt}_{half}")
                            nc.vector.tensor_copy(out=pt[:], in_=pp[:])
                            nc.sync.dma_start(
                                rs_in[u][tt * 128:(tt + 1) * 128,
                                         half * QC:(half + 1) * QC],
                                pt[:])
                    nc.gpsimd.collective_compute(
                        "ReduceScatter",
                        mybir.AluOpType.add,
                        replica_groups=[list(range(NCORES))],
                        ins=[rs_in[u][:]],
                        outs=[rs_out[u][:]],
                    )
                return emit

            def post_closure(u, postpool):
                """Post-RS tail for unit u: bias add + fp32 out rows."""
                def emit():
                    rsb = postpool.tile([64, C], F16, tag="rsb",
                                        name=f"rsb_{u}")
                    nc.sync.dma_start(rsb[:], rs_out[u][:])
                    ob = postpool.tile([64, C], F32, tag="ob",
                                       name=f"ob_{u}")
                    nc.vector.tensor_tensor(ob[:], rsb[:], bias_bc[:],
                                            mybir.AluOpType.add)
                    nc.sync.dma_start(out_ext[u * 64:(u + 1) * 64, :], ob[:])
                return emit

            def attn_phase(spsum, opsum, auxpsum, ppool, denpool, rbpool,
                           onpool, projpool, postpool, bat, pending):
                """pending: deque of closures (cross-batch carryover) run
                one per group boundary; per-unit proj/post closures are
                appended with lag so the PE never waits on normalization."""
                for uu in range(N // QC):
                    u = bat * (N // QC) + uu
                    qcol = u * QC
                    units = [(h, mt) for mt in range(NMT) for h in range(2)]
                    o_cur = {}
                    o_n2 = onpool.tile([128, QC], BF16, tag="on",
                                       name=f"on_{u}")
                    heads_done = [0]
                    for g0 in range(0, len(units), GRP):
                        g = units[g0:g0 + GRP]
                        s_t = spsum.tile([128, GRP, QC], F32, tag="s",
                                         name=f"s_{u}_{g0}")
                        for ui, (h, mt) in enumerate(g):
                            if mt == 0 and h not in o_cur:
                                o_cur[h] = opsum.tile(
                                    [65, QC], F32, tag=f"o{h}",
                                    name=f"o_ps{h}_{u}")
                            nc.tensor.matmul(
                                s_t[:, ui, :],
                                kT_sb[h * 64:(h + 1) * 64,
                                      bat * N + mt * 128:
                                      bat * N + (mt + 1) * 128],
                                qT_sb[h * 64:(h + 1) * 64, qcol:qcol + QC],
                                start=True, stop=True)
                        p_t = ppool.tile([128, GRP, QC], BF16, tag="p",
                                         name=f"p_{u}_{g0}")
                        nc.scalar.activation(p_t[:, 0:len(g), :],
                                             s_t[:, 0:len(g), :],
                                             EXP, scale=SCALE)
                        for ui, (h, mt) in enumerate(g):
                            nc.tensor.matmul(
                                o_cur[h][:],
                                v_sb[:, bat * NMT + mt, h * 65:(h + 1) * 65],
                                p_t[:, ui, :],
                                start=(mt == 0), stop=(mt == NMT - 1))
                            if mt == NMT - 1:
                                o_ps = o_cur.pop(h)
                                nc.vector.tensor_copy(
                                    out=stage[:, u * 2 + h, :],
                                    in_=o_ps[0:65, :])
                                rcp = denpool.tile([1, QC], F32, tag="rcp",
                                                   name=f"rcp_{u}_{h}")
                                nc.vector.reciprocal(
                                    rcp[:], stage[64:65, u * 2 + h, :])
                                rb = rbpool.tile([64, QC], F32, tag="rb",
                                                 name=f"rb_{u}_{h}")
                                nc.gpsimd.partition_broadcast(rb[:], rcp[:])
                                nc.vector.tensor_tensor(
                                    o_n2[h * 64:(h + 1) * 64, :],
                                    stage[0:64, u * 2 + h, :],
                                    rb[:], mybir.AluOpType.mult)
                                heads_done[0] += 1
                                if heads_done[0] == 2:
                                    pending.append(
                                        proj_closure(u, o_n2, auxpsum,
                                                     projpool))
                                    pending.append(post_closure(u, postpool))
                        # drain one pending closure per group, but keep a
                        # lag of >=1 so the PE never waits on the normalize
                        # chain of the unit that just finished
                        if len(pending) > 2:
                            pending.pop(0)()

            with (
                tc.tile_pool(name="xchunk", bufs=1) as xpool,
                tc.tile_pool(name="vtmp", bufs=2) as vpool,
                tc.tile_pool(name="pexp", bufs=4) as ppool,
                tc.tile_pool(name="denp", bufs=4) as denpool,
                tc.tile_pool(name="rbp", bufs=4) as rbpool,
                tc.tile_pool(name="onrm", bufs=3) as onpool,
                tc.tile_pool(name="projp", bufs=4) as projpool,
                tc.tile_pool(name="postp", bufs=2) as postpool,
                tc.tile_pool(name="auxps", bufs=2, space="PSUM") as auxpsum,
            ):
                # batch-0 x chunks load first (fine-grained for fast start);
                # batch-1 chunks as single 3D DMAs
                x_tiles = []
                for nch in range(NT // XCH):
                    x_t = xpool.tile([128, KT, XCH], BF16, tag=f"x{nch}",
                                     name=f"x_{nch}")
                    x_tiles.append(x_t)
                for kt in range(KT):
                    nc.sync.dma_start(x_tiles[0][:, kt, :],
                                      xT_v[:, kt, 0:XCH])
                for nch in range(1, NT // XCH):
                    nc.sync.dma_start(
                        x_tiles[nch][:],
                        xT_v[:, :, nch * XCH:(nch + 1) * XCH])

                pending = []
                for bat in range(B):
                    with (
                        tc.tile_pool(name=f"qkvps{bat}", bufs=2,
                                     space="PSUM") as apsum,
                        tc.tile_pool(name=f"trps{bat}", bufs=2,
                                     space="PSUM") as tpsum,
                    ):
                        for gi, emit in enumerate(qkv_groups(
                                vpool, apsum, tpsum, x_tiles, bat,
                                f"a{bat}")):
                            emit()
                            # carry-over proj/post from the previous batch
                            # drains early in this QKV phase
                            if gi >= 2 and pending:
                                pending.pop(0)()
                    with (
                        tc.tile_pool(name=f"sps{bat}", bufs=2,
                                     space="PSUM") as spsum,
                        tc.tile_pool(name=f"ops{bat}", bufs=1,
                                     space="PSUM") as opsum,
                    ):
                        attn_phase(spsum, opsum, auxpsum, ppool, denpool,
                                   rbpool, onpool, projpool, postpool, bat,
                                   pending)
                        if bat == B - 1:
                            for f in pending:
                                f()
                            pending.clear()
    nc.compile()
    return nc


def kernel(x, w_qkv, w_proj, b_proj):
    global _NC, LAST_EXEC_NS
    if _NC is None:
        _NC = _build()
    x = np.asarray(x, dtype=np.float32)
    w_qkv = np.asarray(w_qkv, dtype=np.float32)
    w_proj = np.asarray(w_proj, dtype=np.float32)
    b_proj = np.asarray(b_proj, dtype=np.float32)

    import ml_dtypes
    xT = np.ascontiguousarray(x.reshape(NT, C).T).astype(ml_dtypes.bfloat16)
    wpT_full = np.ascontiguousarray(w_proj.T)
    bias = np.ascontiguousarray(b_proj.reshape(1, C))
    idn = np.eye(128, dtype=ml_dtypes.bfloat16)
    in_maps = []
    for c in range(NCORES):
        blk = slice(128 * c, 128 * (c + 1))
        wT = np.ascontiguousarray(
            np.concatenate([w_qkv[0:C][blk], w_qkv[C:2 * C][blk],
                            w_qkv[2 * C:3 * C][blk]], axis=0).T).astype(
                ml_dtypes.bfloat16)
        wpT = np.ascontiguousarray(wpT_full[blk]).astype(ml_dtypes.bfloat16)
        in_maps.append({"xT": xT, "wT": wT, "wpT": wpT, "bias": bias,
                        "idn": idn})

    if TRACE:
        _install_ntff_hook()
    res = run_bass_kernel_spmd(_NC, in_maps, core_ids=list(range(NCORES)),
                               trace=TRACE)
    LAST_EXEC_NS = res.exec_time_ns
    # core c's rows are (u, 64) for u=0..7: global token u*512 + c*64 + i
    arr = np.stack([res.results[i]["out"] for i in range(NCORES)])
    out = arr.reshape(NCORES, NU, 64, C).transpose(1, 0, 2, 3)
    return np.ascontiguousarray(
        out.reshape(B, N, C).astype(np.float32))


# revision 16
# speedup vs baseline: 1.0803x; 1.0803x over previous
"""Distributed multi-head attention for 8 TRN2 NeuronCores — v2.

Problem: x[2,2048,1024] -> QKV proj (w_qkv[3072,1024]) -> 16-head SDPA ->
out proj (w_proj[1024,1024] + b_proj) -> [2,2048,1024].

Sharding: 2 heads per core (head-parallel over all 8 cores; both batches on
every core). Per core:
  Phase A: qT/kT [128(=2 heads x 64d), 4096] and V-natural [4096, 128] from
           x @ w_qkv_shard.T; V via PE transpose of vT.
  Phase B: transposed-score attention per (batch, 512-query chunk) u:
           S^T[m,n] = kT.T @ qT (row-tiled K=64 matmul pairs),
           P = exp(S*scale), O^T_ext[65,n] = [V|1].T @ P^T accumulated over
           key tiles in PSUM (row 64 = softmax denominator). Normalized
           output for both heads lands in o_n2[128ch, 512tok].
  Fused out-proj: partial[512tok, 1024f] = o_n2.T @ wpT_shard (my 128
           channels only; K=128, no accumulation) -> fp16 -> rs_in[u].
  Per-unit ReduceScatter #u sums partials over cores and scatters rows:
           core c receives final out rows [u*512 + c*64 .. +64) (minus bias).
           The 8 RS ops overlap with subsequent attention compute.
  Post-RS: add bias, write fp32 out rows. Host reorders rows.
"""
import sys, os, types
import numpy as np

if "/opt/trn_rl_repo" not in sys.path and os.path.isdir("/opt/trn_rl_repo"):
    sys.path.append("/opt/trn_rl_repo")

import concourse.bass as bass
import concourse.mybir as mybir
import concourse.tile as tile
from concourse import bacc
from concourse.bass_utils import run_bass_kernel_spmd

F32 = mybir.dt.float32
F16 = mybir.dt.float16
BF16 = mybir.dt.bfloat16
EXP = mybir.ActivationFunctionType.Exp
ACOPY = mybir.ActivationFunctionType.Copy

NCORES = 8
B, N, C, H, D = 2, 2048, 1024, 16, 64
NT = B * N          # 4096 flat tokens
KT = C // 128       # 8 contraction tiles of 128
QC = 512            # query-chunk width (one PSUM bank)
NU = NT // QC       # 8 (batch, qchunk) units == RS count
NMT = N // 128      # 16 key tiles per batch
SCALE = 1.0 / 8.0   # 1/sqrt(D)
GRP = 2             # score banks per exp call
XCH = 1024          # x load chunk width

TRACE = False
LAST_EXEC_NS = None

_NC = None


def _install_ntff_hook():
    if "antenv.axon_hooks" in sys.modules:
        return
    try:
        import antenv
        from trn_agent_boot.trn_boot import _ntff_profile_via_ctypes
        mod = types.ModuleType("antenv.axon_hooks")
        _hook = [None]
        mod.set_axon_ntff_profile_hook = lambda h: _hook.__setitem__(0, h)
        mod.get_axon_ntff_profile_hook = lambda: _hook[0]
        sys.modules["antenv.axon_hooks"] = mod
        antenv.axon_hooks = mod
        mod.set_axon_ntff_profile_hook(
            _ntff_profile_via_ctypes("/opt/axon/libaxon_pjrt.so"))
    except Exception:
        pass


def _build():
    nc = bacc.Bacc("TRN2", target_bir_lowering=False, debug=False,
                   num_devices=NCORES)
    xT_ext = nc.dram_tensor("xT", [C, NT], BF16, kind="ExternalInput").ap()
    wT_ext = nc.dram_tensor("wT", [C, 384], BF16, kind="ExternalInput").ap()
    wpT_ext = nc.dram_tensor("wpT", [128, C], BF16, kind="ExternalInput").ap()
    bias_ext = nc.dram_tensor("bias", [1, C], F32, kind="ExternalInput").ap()
    idn_ext = nc.dram_tensor("idn", [128, 128], BF16, kind="ExternalInput").ap()
    out_ext = nc.dram_tensor("out", [NT // NCORES, C], F32,
                             kind="ExternalOutput").ap()
    rs_in = [nc.dram_tensor(f"rs_in{u}", [QC, C], F16).ap()
             for u in range(NU)]
    rs_out = [nc.dram_tensor(f"rs_out{u}", [64, C], F16).ap()
              for u in range(NU)]

    xT_v = xT_ext.rearrange("(kt p) n -> p kt n", p=128)
    wT_v = wT_ext.rearrange("(kt p) f -> p kt f", p=128)

    with tile.TileContext(nc) as tc:
        with (
            tc.tile_pool(name="const", bufs=1) as cpool,
            tc.tile_pool(name="resid", bufs=1) as rpool,
        ):
            wT_sb = cpool.tile([128, KT, 384], BF16)
            for kt in range(KT):
                nc.sync.dma_start(wT_sb[:, kt, :], wT_v[:, kt, :])
            idn = cpool.tile([128, 128], BF16)
            nc.sync.dma_start(idn[:], idn_ext[:])
            wp_sb = cpool.tile([128, C], BF16)
            nc.sync.dma_start(wp_sb[:], wpT_ext[:])
            bias_sb = cpool.tile([1, C], F32)
            nc.sync.dma_start(bias_sb[:], bias_ext[:])
            bias_bc = cpool.tile([64, C], F32)
            nc.gpsimd.partition_broadcast(bias_bc[:], bias_sb[:])

            qT_sb = rpool.tile([128, NT], BF16)
            kT_sb = rpool.tile([128, NT], BF16)
            v_sb = rpool.tile([128, NT // 128, 130], BF16)
            nc.gpsimd.memset(v_sb[:, :, 64], 1.0)
            nc.gpsimd.memset(v_sb[:, :, 129], 1.0)
            # unnormalized attention outputs + denominator row: block (u, h)
            # at [0:65, u*2+h, :]; row 64 is the softmax denominator
            stage = rpool.tile([65, 2 * NU, QC], F32)

            def qkv_groups(vpool, apsum, tpsum, x_tiles, bat, psum_tag):
                """Yield one closure per QKV matmul-group (8 accumulating
                matmuls + PSUM evacuation; the v-feature groups also emit
                the PE transposes building V-natural)."""
                for nch2 in range(N // XCH):
                    x_t = x_tiles[bat * (N // XCH) + nch2]
                    for hw in range(XCH // QC):
                        ncol = bat * N + nch2 * XCH + hw * QC
                        for ft in range(3):
                            def emit(ncol=ncol, hw=hw, ft=ft, x_t=x_t):
                                xs = x_t[:, :, hw * QC:(hw + 1) * QC]
                                ps = apsum.tile([128, QC], F32, tag=psum_tag,
                                                name=f"qkv_{ncol}_{ft}")
                                for kt in range(KT):
                                    nc.tensor.matmul(
                                        ps[:],
                                        wT_sb[:, kt, ft * 128:(ft + 1) * 128],
                                        xs[:, kt, :],
                                        start=(kt == 0), stop=(kt == KT - 1))
                                if ft == 0:
                                    nc.vector.tensor_copy(
                                        out=qT_sb[:, ncol:ncol + QC],
                                        in_=ps[:])
                                elif ft == 1:
                                    nc.vector.tensor_copy(
                                        out=kT_sb[:, ncol:ncol + QC],
                                        in_=ps[:])
                                else:
                                    vt = vpool.tile([128, QC], BF16, tag="vt",
                                                    name=f"vt_{ncol}")
                                    nc.vector.tensor_copy(out=vt[:],
                                                          in_=ps[:])
                                    mtg0 = ncol // 128
                                    trp = tpsum.tile(
                                        [128, 4, 128], BF16, tag="tr",
                                        name=f"tr_{mtg0}")
                                    for t in range(4):
                                        nc.tensor.transpose(
                                            trp[:, t, :],
                                            vt[:, t * 128:(t + 1) * 128],
                                            idn[:])
                                    nc.vector.tensor_copy(
                                        out=v_sb[:, mtg0:mtg0 + 4, 0:64],
                                        in_=trp[:, :, 0:64])
                                    nc.vector.tensor_copy(
                                        out=v_sb[:, mtg0:mtg0 + 4, 65:129],
                                        in_=trp[:, :, 64:128])
                            yield emit

            def proj_closure(u, o_n2, auxpsum, projpool):
                """Fused out-projection for unit u: partial[tok,f] from my
                128 channels; fp16 to rs_in[u]; then the ReduceScatter."""
                def emit():
                    for tt in range(4):
                        for half in range(2):
                            pp = auxpsum.tile([128, QC], F32, tag="pp",
                                              name=f"pp_{u}_{tt}_{half}")
                            nc.tensor.matmul(
                                pp[:],
                                o_n2[:, tt * 128:(tt + 1) * 128],
                                wp_sb[:, half * QC:(half + 1) * QC],
                                start=True, stop=True)
                            pt = projpool.tile([128, QC], F16, tag="pt",
                                               name=f"pt_{u}_{tt}_{half}")
                            nc.scalar.activation(pt[:], pp[:], ACOPY)
                            nc.sync.dma_start(
                                rs_in[u][tt * 128:(tt + 1) * 128,
                                         half * QC:(half + 1) * QC],
                                pt[:])
                    nc.gpsimd.collective_compute(
                        "ReduceScatter",
                        mybir.AluOpType.add,
                        replica_groups=[list(range(NCORES))],
                        ins=[rs_in[u][:]],
                        outs=[rs_out[u][:]],
                    )
                return emit

            def post_closure(u, postpool):
                """Post-RS tail for unit u: bias add + fp32 out rows."""
                def emit():
                    rsb = postpool.tile([64, C], F16, tag="rsb",
                                        name=f"rsb_{u}")
                    nc.sync.dma_start(rsb[:], rs_out[u][:])
                    ob = postpool.tile([64, C], F32, tag="ob",
                                       name=f"ob_{u}")
                    nc.vector.tensor_tensor(ob[:], rsb[:], bias_bc[:],
                                            mybir.AluOpType.add)
                    nc.sync.dma_start(out_ext[u * 64:(u + 1) * 64, :], ob[:])
                return emit

            def attn_phase(spsum, opsum, auxpsum, ppool, denpool, rbpool,
                           onpool, projpool, postpool, bat, pend_proj,
                           pend_post):
                """pend_proj: ~1-unit lag so the PE never waits on the
                normalize chain. pend_post: ~5-unit lag so the DVE/sync
                streams never head-of-line block on an in-flight RS."""
                for uu in range(N // QC):
                    u = bat * (N // QC) + uu
                    qcol = u * QC
                    units = [(h, mt) for mt in range(NMT) for h in range(2)]
                    o_cur = {}
                    o_n2 = onpool.tile([128, QC], BF16, tag="on",
                                       name=f"on_{u}")
                    heads_done = [0]
                    for g0 in range(0, len(units), GRP):
                        g = units[g0:g0 + GRP]
                        s_t = spsum.tile([128, GRP, QC], F32, tag="s",
                                         name=f"s_{u}_{g0}")
                        for ui, (h, mt) in enumerate(g):
                            if mt == 0 and h not in o_cur:
                                o_cur[h] = opsum.tile(
                                    [65, QC], F32, tag=f"o{h}",
                                    name=f"o_ps{h}_{u}")
                            nc.tensor.matmul(
                                s_t[:, ui, :],
                                kT_sb[h * 64:(h + 1) * 64,
                                      bat * N + mt * 128:
                                      bat * N + (mt + 1) * 128],
                                qT_sb[h * 64:(h + 1) * 64, qcol:qcol + QC],
                                start=True, stop=True)
                        p_t = ppool.tile([128, GRP, QC], BF16, tag="p",
                                         name=f"p_{u}_{g0}")
                        nc.scalar.activation(p_t[:, 0:len(g), :],
                                             s_t[:, 0:len(g), :],
                                             EXP, scale=SCALE)
                        for ui, (h, mt) in enumerate(g):
                            nc.tensor.matmul(
                                o_cur[h][:],
                                v_sb[:, bat * NMT + mt, h * 65:(h + 1) * 65],
                                p_t[:, ui, :],
                                start=(mt == 0), stop=(mt == NMT - 1))
                            if mt == NMT - 1:
                                o_ps = o_cur.pop(h)
                                nc.vector.tensor_copy(
                                    out=stage[:, u * 2 + h, :],
                                    in_=o_ps[0:65, :])
                                rcp = denpool.tile([1, QC], F32, tag="rcp",
                                                   name=f"rcp_{u}_{h}")
                                nc.vector.reciprocal(
                                    rcp[:], stage[64:65, u * 2 + h, :])
                                rb = rbpool.tile([64, QC], F32, tag="rb",
                                                 name=f"rb_{u}_{h}")
                                nc.gpsimd.partition_broadcast(rb[:], rcp[:])
                                nc.vector.tensor_tensor(
                                    o_n2[h * 64:(h + 1) * 64, :],
                                    stage[0:64, u * 2 + h, :],
                                    rb[:], mybir.AluOpType.mult)
                                heads_done[0] += 1
                                if heads_done[0] == 2:
                                    pend_proj.append(
                                        proj_closure(u, o_n2, auxpsum,
                                                     projpool))
                                    pend_post.append(post_closure(u,
                                                                  postpool))
                        # drain with lag: proj one unit behind; post five
                        # units behind (RS#u must have completed)
                        if len(pend_proj) > 1:
                            pend_proj.pop(0)()
                        if len(pend_post) > 5:
                            pend_post.pop(0)()

            with (
                tc.tile_pool(name="xchunk", bufs=1) as xpool,
                tc.tile_pool(name="vtmp", bufs=2) as vpool,
                tc.tile_pool(name="pexp", bufs=4) as ppool,
                tc.tile_pool(name="denp", bufs=4) as denpool,
                tc.tile_pool(name="rbp", bufs=4) as rbpool,
                tc.tile_pool(name="onrm", bufs=3) as onpool,
                tc.tile_pool(name="projp", bufs=4) as projpool,
                tc.tile_pool(name="postp", bufs=2) as postpool,
                tc.tile_pool(name="auxps", bufs=2, space="PSUM") as auxpsum,
            ):
                # batch-0 x chunks load first (fine-grained for fast start);
                # batch-1 chunks as single 3D DMAs
                x_tiles = []
                for nch in range(NT // XCH):
                    x_t = xpool.tile([128, KT, XCH], BF16, tag=f"x{nch}",
                                     name=f"x_{nch}")
                    x_tiles.append(x_t)
                for kt in range(KT):
                    nc.sync.dma_start(x_tiles[0][:, kt, :],
                                      xT_v[:, kt, 0:XCH])
                for nch in range(1, NT // XCH):
                    nc.sync.dma_start(
                        x_tiles[nch][:],
                        xT_v[:, :, nch * XCH:(nch + 1) * XCH])

                pend_proj = []
                pend_post = []
                for bat in range(B):
                    with (
                        tc.tile_pool(name=f"qkvps{bat}", bufs=2,
                                     space="PSUM") as apsum,
                        tc.tile_pool(name=f"trps{bat}", bufs=2,
                                     space="PSUM") as tpsum,
                    ):
                        for gi, emit in enumerate(qkv_groups(
                                vpool, apsum, tpsum, x_tiles, bat,
                                f"a{bat}")):
                            emit()
                            # carry-over proj from the previous batch drains
                            # early in this QKV phase (RS trigger asap)
                            if gi >= 2 and pend_proj:
                                pend_proj.pop(0)()
                    with (
                        tc.tile_pool(name=f"sps{bat}", bufs=2,
                                     space="PSUM") as spsum,
                        tc.tile_pool(name=f"ops{bat}", bufs=1,
                                     space="PSUM") as opsum,
                    ):
                        attn_phase(spsum, opsum, auxpsum, ppool, denpool,
                                   rbpool, onpool, projpool, postpool, bat,
                                   pend_proj, pend_post)
                        if bat == B - 1:
                            for f in pend_proj:
                                f()
                            pend_proj.clear()
                            for f in pend_post:
                                f()
                            pend_post.clear()
    nc.compile()
    return nc


def kernel(x, w_qkv, w_proj, b_proj):
    global _NC, LAST_EXEC_NS
    if _NC is None:
        _NC = _build()
    x = np.asarray(x, dtype=np.float32)
    w_qkv = np.asarray(w_qkv, dtype=np.float32)
    w_proj = np.asarray(w_proj, dtype=np.float32)
    b_proj = np.asarray(b_proj, dtype=np.float32)

    import ml_dtypes
    xT = np.ascontiguousarray(x.reshape(NT, C).T).astype(ml_dtypes.bfloat16)
    wpT_full = np.ascontiguousarray(w_proj.T)
    bias = np.ascontiguousarray(b_proj.reshape(1, C))
    idn = np.eye(128, dtype=ml_dtypes.bfloat16)
    in_maps = []
    for c in range(NCORES):
        blk = slice(128 * c, 128 * (c + 1))
        wT = np.ascontiguousarray(
            np.concatenate([w_qkv[0:C][blk], w_qkv[C:2 * C][blk],
                            w_qkv[2 * C:3 * C][blk]], axis=0).T).astype(
                ml_dtypes.bfloat16)
        wpT = np.ascontiguousarray(wpT_full[blk]).astype(ml_dtypes.bfloat16)
        in_maps.append({"xT": xT, "wT": wT, "wpT": wpT, "bias": bias,
                        "idn": idn})

    if TRACE:
        _install_ntff_hook()
    res = run_bass_kernel_spmd(_NC, in_maps, core_ids=list(range(NCORES)),
                               trace=TRACE)
    LAST_EXEC_NS = res.exec_time_ns
    # core c's rows are (u, 64) for u=0..7: global token u*512 + c*64 + i
    arr = np.stack([res.results[i]["out"] for i in range(NCORES)])
    out = arr.reshape(NCORES, NU, 64, C).transpose(1, 0, 2, 3)
    return np.ascontiguousarray(
        out.reshape(B, N, C).astype(np.float32))
